# revision 23
# baseline (speedup 1.0000x reference)
"""Barycentric-coordinates KNN kernel for Trainium2 (8 NeuronCores).

Pipeline (per core = one (batch, half-of-V) pair; 8 cores cover 4 batches x 2 halves):
  Phase 1 (device): per query row, keys -S = 2q.p - |p|^2 (descending rank ==
    ascending d^2) via fp32r (tf32) TensorE matmul; low-7 mantissa bits of the
    PSUM result replaced with the chunk-local index (DVE bitpack STT straight
    from PSUM); DVE max8 per 128-column chunk -> 256 packed candidates/row.
  Host: decode (chunk, k) from packed keys, exact fp64 d^2 re-rank to top-33
    (value asc, index asc), neighbor-coordinate gather, SHOT weight norm.
  Phase 2 (device): weighted 3x3 covariance (fused multiply-accumulate),
    closed-form eigensolver (Newton on the characteristic cubic + cross
    products), SHOT sign disambiguation, tangent-plane log map, template-cell
    nearest-3 selection via bit-packed keys (dist^2 mantissa | k-slot) and
    max8. Ships packed keys + projections.
  Host: decode k-slots, gather projections, barycentric weights,
    pidx = nbr_idx[closest], assemble (4, 4096, 5, 8, 3, 2) output.
"""
import sys

sys.path.insert(0, "/opt/trn_rl_repo")

import numpy as np
from contextlib import ExitStack

import concourse.bass as bass
import concourse.mybir as mybir
import concourse.tile as tile
from concourse.bass_utils import run_bass_kernel_spmd
from concourse.tile import ScopedClock

f32 = np.float32
AF = mybir.ActivationFunctionType
ALU = mybir.AluOpType
DT = mybir.dt

B, V, K = 4, 4096, 32
HALF = V // 2            # queries per core
NT = HALF // 128         # 16 v-tiles per core
CW = 128                 # phase-1 chunk width
NCHUNK = V // CW         # 32 chunks
CAND = NCHUNK * 8        # 256 candidates per row
R, A = 5, 8
NCELL = R * A            # 40 template cells
EPS = 1e-8

# ---------------------------------------------------------------------------
# Tile-framework workaround: walrus rejects instructions carrying more than a
# couple of sync waits. Spread extras across single-wait NOPs.
# ---------------------------------------------------------------------------


def _patched_drain_and_barrier(self, tick_clock, wait_clock):
    probe = self.nc.sync.nop(nofuse=True)
    wait_clock.add_sem_waits(probe.ins, ScopedClock({None: tick_clock.global_clock}))
    sync_info = probe.ins.sync_info
    waits = list(sync_info.on_wait or []) if sync_info is not None else []
    if len(waits) > 1:
        sync_info.on_wait = waits[:1]
        for i in range(1, len(waits)):
            extra = self.nc.sync.nop(nofuse=True)
            if extra.ins.sync_info is None:
                extra.ins.sync_info = mybir.SyncInfo(on_wait=[waits[i]], on_update=[])
            else:
                extra.ins.sync_info.on_wait = [waits[i]]
    self.nc.sync.drain()
    self.nc.all_engine_barrier()
    assert self.sems is not None
    popped = self.nc._tile_sem_poison_stack.pop()
    assert popped is self._sem_poison
    self.nc.clear_and_free_semaphores(list(self.sems.allocated().values()))
    self.nc.all_engine_barrier()


tile.TileContext._drain_and_barrier = _patched_drain_and_barrier


def split_sync_waits(nc, max_waits=1):
    for f in nc.m.functions:
        for b in f.blocks:
            new_list = []
            dirty = False
            for ins in b.instructions:
                si = ins.sync_info
                waits = list(si.on_wait) if (si is not None and si.on_wait) else []
                if len(waits) > max_waits:
                    dirty = True
                    extras, keep = waits[:-max_waits], waits[-max_waits:]
                    for j in range(0, len(extras), max_waits):
                        nop = mybir.InstNoOp(
                            name=f"I-wsplit-{nc.next_id()}", engine=ins.engine
                        )
                        nop.sync_info = mybir.SyncInfo(
                            on_wait=extras[j : j + max_waits], on_update=[]
                        )
                        new_list.append(nop)
                    si.on_wait = keep
                new_list.append(ins)
            if dirty:
                b.instructions = new_list


# ---------------------------------------------------------------------------
# Phase 1 program
# ---------------------------------------------------------------------------


def build_phase1():
    nc = bass.Bass()
    pt4 = nc.declare_dram_parameter("pt4", [4, V], DT.float32r, isOutput=False)
    qt4 = nc.declare_dram_parameter("qt4", [4, HALF], DT.float32r, isOutput=False)
    candv_o = nc.declare_dram_parameter("candv", [HALF, CAND], DT.float32, isOutput=True)

    with tile.TileContext(nc) as tc, ExitStack() as ctx:
        cpool = ctx.enter_context(tc.tile_pool(name="const", bufs=1))
        npool = ctx.enter_context(tc.tile_pool(name="nkey", bufs=3))
        opool = ctx.enter_context(tc.tile_pool(name="cand", bufs=4))
        ppool = ctx.enter_context(tc.tile_pool(name="psum", bufs=2, space="PSUM"))

        pt = cpool.tile([4, V], DT.float32r)
        qt = cpool.tile([4, HALF], DT.float32r)
        nc.sync.dma_start(pt[:], pt4[:])
        nc.sync.dma_start(qt[:], qt4[:])

        KI = cpool.tile([128, 2048], DT.int32)
        nc.gpsimd.iota(KI[:], pattern=[[0, 2048 // CW], [1, CW]], base=0,
                       channel_multiplier=0)
        M128 = cpool.tile([128, 1], DT.int32)
        nc.vector.memset(M128[:], -CW)
        C128 = cpool.tile([128, 1], DT.int32)
        nc.vector.memset(C128[:], CW)

        # Software-pipelined: per tile, jh0 keys are bitpacked on the vector
        # engine (PSUM-direct STT) and consumed by MAX8 immediately; jh1 keys
        # take the slow path (ACT int-quantize -> gpsimd *CW -> gpsimd +iota,
        # ~10us latency) so their MAX8s are deferred one tile to keep the
        # vector engine from stalling on gpsimd.
        NC2 = 2048 // CW
        cvs = [None] * NT
        nks = [None] * NT

        def emit_jh1_max8(t):
            cv, nk1 = cvs[t], nks[t]
            for c in range(NC2):
                cc = NC2 + c
                nc.vector.max(out=cv[:, cc * 8:(cc + 1) * 8],
                              in_=nk1[:, c * CW:(c + 1) * CW].bitcast(DT.float32))
            nc.sync.dma_start(candv_o[t * 128:(t + 1) * 128, :], cv[:])

        for t in range(NT):
            cv = opool.tile([128, CAND], DT.float32, tag="cv")
            cvs[t] = cv
            for jh in range(2):
                ps = ppool.tile([128, 2048], DT.float32, space="PSUM")
                for k4 in range(4):
                    nc.tensor.matmul(
                        ps[:, k4 * 512:(k4 + 1) * 512],
                        qt[:, t * 128:(t + 1) * 128],
                        pt[:, jh * 2048 + k4 * 512: jh * 2048 + (k4 + 1) * 512],
                        start=True, stop=True,
                    )
                nk = npool.tile([128, 2048], DT.int32, tag=f"nk{jh}")
                if jh == 0:
                    nc.vector.scalar_tensor_tensor(
                        out=nk[:], in0=ps[:].bitcast(DT.int32), scalar=M128[:],
                        in1=KI[:], op0=ALU.bitwise_and, op1=ALU.bitwise_or)
                    for c in range(NC2):
                        nc.vector.max(out=cv[:, c * 8:(c + 1) * 8],
                                      in_=nk[:, c * CW:(c + 1) * CW].bitcast(DT.float32))
                else:
                    # gpsimd has no bitwise ops and can't read PSUM. The idle
                    # ACT engine quantizes to a positive int32 rank key
                    # (monotone in the fp32 value, ~5e-7 resolution), gpsimd
                    # shifts it by the chunk width and adds the local index.
                    # The bit layout (v << 7 | k) matches the jh0 half, so
                    # host decode is identical.
                    qv = npool.tile([128, 2048], DT.int32, tag="quant")
                    nc.scalar.activation(qv[:], ps[:], AF.Copy,
                                         bias=4.0 * 2097152.0, scale=2097152.0)
                    sc128 = npool.tile([128, 2048], DT.int32, tag="scaled")
                    nc.gpsimd.tensor_tensor(
                        out=sc128[:], in0=qv[:],
                        in1=C128[:].to_broadcast([128, 2048]), op=ALU.mult)
                    nc.gpsimd.tensor_tensor(out=nk[:], in0=sc128[:], in1=KI[:],
                                            op=ALU.add)
                    nks[t] = nk
            if t > 0:
                emit_jh1_max8(t - 1)
        emit_jh1_max8(NT - 1)

    split_sync_waits(nc)
    return nc


# ---------------------------------------------------------------------------
# Phase 2 program
# ---------------------------------------------------------------------------


def _register_consts(nc, values):
    for value in values:
        t = nc.alloc_sbuf_tensor(f"const-float32-{value}", [128, 1], DT.float32)
        nc.gpsimd.memset(t.ap(), value)
        nc.const_aps.aps[(DT.float32, value)] = t.ap()
    nc.all_engine_barrier()


def build_phase2():
    nc = bass.Bass()
    _register_consts(nc, [0.5])
    ngh_i = nc.declare_dram_parameter("ngh", [HALF, 96], DT.float32, isOutput=False)
    wn3_i = nc.declare_dram_parameter("wn3", [HALF, 96], DT.float32, isOutput=False)
    dd_i = nc.declare_dram_parameter("dd", [HALF, K], DT.float32, isOutput=False)
    txy_i = nc.declare_dram_parameter("txy", [128, 2 * NCELL], DT.float32, isOutput=False)
    m3_o = nc.declare_dram_parameter("m3o", [HALF, NCELL, 3], DT.float32, isOutput=True)
    pxy_o = nc.declare_dram_parameter("pxy", [HALF, 2 * K], DT.float32, isOutput=True)

    with tile.TileContext(nc) as tc, ExitStack() as ctx:
        cp = ctx.enter_context(tc.tile_pool(name="const", bufs=1))
        sp = ctx.enter_context(tc.tile_pool(name="scratch", bufs=2))
        bp = ctx.enter_context(tc.tile_pool(name="bc", bufs=2))

        NGH = cp.tile([128, NT, 96], DT.float32)
        WN3 = cp.tile([128, NT, 96], DT.float32)
        DD = cp.tile([128, NT, K], DT.float32)
        TXY = cp.tile([128, 2 * NCELL], DT.float32)
        nc.sync.dma_start(NGH[:], ngh_i[:].rearrange("(t p) c -> p t c", p=128))
        nc.sync.dma_start(WN3[:], wn3_i[:].rearrange("(t p) c -> p t c", p=128))
        nc.sync.dma_start(DD[:], dd_i[:].rearrange("(t p) c -> p t c", p=128))
        nc.sync.dma_start(TXY[:], txy_i[:])
        TX = TXY[:, 0:NCELL]
        TY = TXY[:, NCELL:2 * NCELL]

        KIOTA = cp.tile([128, NCELL, K], DT.int32)
        nc.gpsimd.iota(KIOTA[:], pattern=[[0, NCELL], [1, K]], base=-2147483648,
                       channel_multiplier=0)
        M32 = cp.tile([128, 1], DT.int32)
        nc.vector.memset(M32[:], -32)
        # materialized template broadcasts: contiguous in1 reads in the
        # per-tile keys loop instead of stride-0 APs
        TXM = cp.tile([128, NCELL, K], DT.float32)
        TYM = cp.tile([128, NCELL, K], DT.float32)
        nc.vector.tensor_copy(
            TXM[:], TX.rearrange("p r -> p r ()").to_broadcast([128, NCELL, K]))
        nc.vector.tensor_copy(
            TYM[:], TY.rearrange("p r -> p r ()").to_broadcast([128, NCELL, K]))

        _tagn = [0]

        def nt_tile(pool=cp):
            _tagn[0] += 1
            return pool.tile([128, NT], DT.float32, tag=f"nt{_tagn[0]}",
                             name=f"nt{_tagn[0]}")

        # ---- covariance accumulation (batched across all v-tiles) ----
        CXX, CXY, CXZ, CYY, CYZ, CZZ = [nt_tile() for _ in range(6)]
        cov_dsts = {"xx": CXX, "xy": CXY, "xz": CXZ, "yy": CYY, "yz": CYZ, "zz": CZZ}
        pairs = [("xx", 0, 0), ("xy", 0, 1), ("xz", 0, 2),
                 ("yy", 1, 1), ("yz", 1, 2), ("zz", 2, 2)]
        NW = sp.tile([128, NT, 96], DT.float32, tag="nw")
        nc.vector.tensor_tensor(out=NW[:], in0=NGH[:], in1=WN3[:], op=ALU.mult)
        for nmq, a, b in pairs:
            prd = sp.tile([128, NT, K], DT.float32, tag="covp")
            nc.vector.tensor_tensor(out=prd[:], in0=NGH[:, :, a * K:(a + 1) * K],
                                    in1=NW[:, :, b * K:(b + 1) * K], op=ALU.mult)
            nc.vector.tensor_reduce(out=cov_dsts[nmq][:], in_=prd[:],
                                    axis=mybir.AxisListType.X, op=ALU.add)

        # ---- eigensolver on (128, NT) ----
        def _ap(x):
            return x if isinstance(x, bass.AP) else x[:]

        def tt(dst, a, bb, op):
            nc.vector.tensor_tensor(out=_ap(dst), in0=_ap(a), in1=_ap(bb), op=op)

        def sq_act(dst, a):
            # vector TT square: keeps the serial eigensolver chain on one
            # engine (ACT round-trips on [128,16] tiles cost ~600ns latency
            # each and stall the chain)
            nc.vector.tensor_tensor(out=dst[:], in0=a[:], in1=a[:], op=ALU.mult)

        Q = nt_tile()
        tt(Q, CXX, CYY, ALU.add)
        tt(Q, Q, CZZ, ALU.add)
        nc.vector.tensor_scalar_mul(Q[:], Q[:], 1.0 / 3.0)
        BXX, BYY, BZZ = nt_tile(), nt_tile(), nt_tile()
        tt(BXX, CXX, Q, ALU.subtract)
        tt(BYY, CYY, Q, ALU.subtract)
        tt(BZZ, CZZ, Q, ALU.subtract)
        P2 = nt_tile()
        T1 = nt_tile(sp)
        sq_act(P2, BXX)
        sq_act(T1, BYY)
        tt(P2, P2, T1, ALU.add)
        sq_act(T1, BZZ)
        tt(P2, P2, T1, ALU.add)
        T2 = nt_tile(sp)
        sq_act(T1, CXY)
        sq_act(T2, CXZ)
        tt(T1, T1, T2, ALU.add)
        sq_act(T2, CYZ)
        tt(T1, T1, T2, ALU.add)
        nc.vector.tensor_scalar_mul(T1[:], T1[:], 2.0)
        tt(P2, P2, T1, ALU.add)
        PP = nt_tile()
        PPX = nt_tile()
        nc.vector.tensor_scalar_mul(PPX[:], P2[:], 1.0 / 6.0)

        def polished_sqrt(dst, x, tmp):
            # ACT Sqrt is ~7e-6; one Newton step s' = (s + x/s)/2 fixes it
            nc.scalar.activation(dst[:], x[:], AF.Sqrt)
            nc.vector.tensor_scalar_max(tmp[:], dst[:], 1e-30)
            nc.vector.reciprocal(tmp[:], tmp[:])
            nc.vector.tensor_tensor(out=tmp[:], in0=x[:], in1=tmp[:], op=ALU.mult)
            nc.vector.tensor_tensor(out=dst[:], in0=dst[:], in1=tmp[:], op=ALU.add)
            nc.vector.tensor_scalar_mul(dst[:], dst[:], 0.5)

        polished_sqrt(PP, PPX, T2)
        PINV = nt_tile()
        nc.vector.tensor_scalar_max(PINV[:], PP[:], 1e-20)
        nc.vector.reciprocal(PINV[:], PINV[:])
        NBXX, NBYY, NBZZ, NBXY, NBXZ, NBYZ = [nt_tile() for _ in range(6)]
        tt(NBXX, BXX, PINV, ALU.mult)
        tt(NBYY, BYY, PINV, ALU.mult)
        tt(NBZZ, BZZ, PINV, ALU.mult)
        tt(NBXY, CXY, PINV, ALU.mult)
        tt(NBXZ, CXZ, PINV, ALU.mult)
        tt(NBYZ, CYZ, PINV, ALU.mult)
        # det(B̂)
        DET = nt_tile()
        sq_act(T1, NBYZ)                     # byz^2
        tt(T2, NBYY, NBZZ, ALU.mult)
        tt(T2, T2, T1, ALU.subtract)
        tt(DET, NBXX, T2, ALU.mult)          # + bxx (byy bzz - byz^2)
        tt(T1, NBXY, NBZZ, ALU.mult)
        tt(T2, NBYZ, NBXZ, ALU.mult)
        tt(T1, T1, T2, ALU.subtract)
        tt(T1, NBXY, T1, ALU.mult)
        tt(DET, DET, T1, ALU.subtract)       # - bxy (bxy bzz - byz bxz)
        tt(T1, NBXY, NBYZ, ALU.mult)
        tt(T2, NBYY, NBXZ, ALU.mult)
        tt(T1, T1, T2, ALU.subtract)
        tt(T1, NBXZ, T1, ALU.mult)
        tt(DET, DET, T1, ALU.add)            # + bxz (bxy byz - byy bxz)
        R2 = nt_tile()                       # 2r = det  clamped to [-2, 2]
        nc.vector.tensor_scalar_min(R2[:], DET[:], 2.0)
        nc.vector.tensor_scalar_max(R2[:], R2[:], -2.0)

        # Both Newton chains (min/max root) and both eigenvector extractions
        # run identical code on different data; batch each pair into
        # [128, 2, NT] tiles (slice 0 = min root / Z axis, 1 = max / X).
        _tag2 = [0]

        def nt2_tile(pool=cp):
            _tag2[0] += 1
            return pool.tile([128, 2, NT], DT.float32, tag=f"n2_{_tag2[0]}",
                             name=f"n2_{_tag2[0]}")

        def b2(x):
            return _ap(x).rearrange("p t -> p () t").to_broadcast([128, 2, NT])

        BETA = nt2_tile()
        nc.vector.memset(BETA[:, 0, :], -2.2)
        nc.vector.memset(BETA[:, 1, :], 2.2)
        R2B = b2(R2)
        FV = nt2_tile(sp)
        B2T = nt2_tile(sp)
        T1B = nt2_tile(sp)
        for _ in range(8):
            sq_act(B2T, BETA)                             # β²
            tt(FV, B2T, BETA, ALU.mult)                   # β³
            nc.vector.scalar_tensor_tensor(
                out=T1B[:], in0=BETA[:], scalar=3.0, in1=FV[:],
                op0=ALU.mult, op1=ALU.subtract)           # 3β - β³
            tt(T1B, T1B, R2B, ALU.add)                    # 3β - β³ + 2r = -f
            nc.vector.tensor_scalar(out=B2T[:], in0=B2T[:], scalar1=3.0,
                                    scalar2=-3.0, op0=ALU.mult, op1=ALU.add)  # f' = 3β²-3
            nc.vector.tensor_scalar_max(B2T[:], B2T[:], 1e-8)
            nc.vector.reciprocal(B2T[:], B2T[:])
            tt(T1B, T1B, B2T, ALU.mult)                   # -f/f'
            tt(BETA, BETA, T1B, ALU.add)                  # β - f/f'
        LAM = nt2_tile()
        tt(LAM, b2(PP), BETA, ALU.mult)
        tt(LAM, LAM, b2(Q), ALU.add)

        # columns of A - lam I (cov components broadcast over the pair dim)
        D0, D1, D2 = nt2_tile(sp), nt2_tile(sp), nt2_tile(sp)
        tt(D0, b2(CXX), LAM, ALU.subtract)
        tt(D1, b2(CYY), LAM, ALU.subtract)
        tt(D2, b2(CZZ), LAM, ALU.subtract)
        m0 = (D0, b2(CXY), b2(CXZ))
        m1 = (b2(CXY), D1, b2(CYZ))
        m2 = (b2(CXZ), b2(CYZ), D2)

        def cross(u, v):
            rx, ry, rz = nt2_tile(sp), nt2_tile(sp), nt2_tile(sp)
            tt(rx, u[1], v[2], ALU.mult)
            tt(T1B, u[2], v[1], ALU.mult)
            tt(rx, rx, T1B, ALU.subtract)
            tt(ry, u[2], v[0], ALU.mult)
            tt(T1B, u[0], v[2], ALU.mult)
            tt(ry, ry, T1B, ALU.subtract)
            tt(rz, u[0], v[1], ALU.mult)
            tt(T1B, u[1], v[0], ALU.mult)
            tt(rz, rz, T1B, ALU.subtract)
            return rx, ry, rz

        def norm2(c):
            n = nt2_tile(sp)
            sq_act(n, c[0])
            sq_act(T1B, c[1])
            tt(n, n, T1B, ALU.add)
            sq_act(T1B, c[2])
            tt(n, n, T1B, ALU.add)
            return n

        c01 = cross(m0, m1)
        c02 = cross(m0, m2)
        c12 = cross(m1, m2)
        n01, n02, n12 = norm2(c01), norm2(c02), norm2(c12)
        G1, G2, G3 = nt2_tile(sp), nt2_tile(sp), nt2_tile(sp)
        tt(G1, n01, n02, ALU.is_ge)
        tt(G2, n01, n12, ALU.is_ge)
        tt(G1, G1, G2, ALU.mult)                    # pick01
        tt(G3, n02, n12, ALU.is_ge)
        U = nt2_tile(sp)
        nc.vector.tensor_scalar(out=U[:], in0=G1[:], scalar1=-1.0, scalar2=1.0,
                                op0=ALU.mult, op1=ALU.add)   # 1 - pick01
        tt(G2, U, G3, ALU.mult)                     # pick02
        nc.vector.tensor_scalar(out=G3[:], in0=G3[:], scalar1=-1.0, scalar2=1.0,
                                op0=ALU.mult, op1=ALU.add)   # 1 - g3
        tt(G3, U, G3, ALU.mult)                     # pick12
        EV = []
        for ci in range(3):
            VC = nt2_tile()
            tt(VC, c01[ci], G1, ALU.mult)
            tt(T1B, c02[ci], G2, ALU.mult)
            tt(VC, VC, T1B, ALU.add)
            tt(T1B, c12[ci], G3, ALU.mult)
            tt(VC, VC, T1B, ALU.add)
            EV.append(VC)
        n2v = norm2(EV)
        nrm = nt2_tile(sp)
        polished_sqrt(nrm, n2v, T1B)
        nc.vector.tensor_scalar_max(nrm[:], nrm[:], 1e-30)
        nc.vector.reciprocal(nrm[:], nrm[:])
        for VC in EV:
            tt(VC, VC, nrm, ALU.mult)
        ZAX = [EV[c][:, 0, :] for c in range(3)]
        XAX = [EV[c][:, 1, :] for c in range(3)]

        # ---- disambiguation dots (batched: axis scalar broadcast over k) ----
        def batched_dot(DST, AX):
            T = sp.tile([128, NT, K], DT.float32, tag="dotT")
            for c in range(3):
                axb = _ap(AX[c]).rearrange("p t -> p t ()").to_broadcast([128, NT, K])
                dst = DST[:] if c == 0 else T[:]
                nc.vector.tensor_tensor(out=dst, in0=NGH[:, :, c * K:(c + 1) * K],
                                        in1=axb, op=ALU.mult)
                if c > 0:
                    nc.vector.tensor_tensor(out=DST[:], in0=DST[:], in1=T[:],
                                            op=ALU.add)

        DOTX = cp.tile([128, NT, K], DT.float32)
        DOTZ = cp.tile([128, NT, K], DT.float32)
        batched_dot(DOTX, XAX)
        batched_dot(DOTZ, ZAX)

        SG = cp.tile([128, NT, K], DT.float32)
        FX = nt_tile()
        FZ = nt_tile()
        for DOT, F in ((DOTX, FX), (DOTZ, FZ)):
            nc.scalar.activation(SG[:], DOT[:], AF.Sign)
            nc.vector.tensor_reduce(out=F[:], in_=SG[:], axis=mybir.AxisListType.X,
                                    op=ALU.add)
            nc.scalar.activation(F[:], F[:], AF.Sign, bias=0.5, scale=1.0)
        for c in range(3):
            tt(XAX[c], XAX[c], FX, ALU.mult)
            tt(ZAX[c], ZAX[c], FZ, ALU.mult)
        fxb = FX[:].rearrange("p t -> p t ()").to_broadcast([128, NT, K])
        nc.vector.tensor_tensor(out=DOTX[:], in0=DOTX[:], in1=fxb, op=ALU.mult)
        # y = cross(z, x)
        YAX = []
        for (i1, i2) in ((1, 2), (2, 0), (0, 1)):
            YC = nt_tile()
            tt(YC, ZAX[i1], XAX[i2], ALU.mult)
            tt(T1, ZAX[i2], XAX[i1], ALU.mult)
            tt(YC, YC, T1, ALU.subtract)
            YAX.append(YC)
        DOTY = cp.tile([128, NT, K], DT.float32)
        batched_dot(DOTY, YAX)

        # ---- projections (batched over all tiles) ----
        PX = cp.tile([128, NT, K], DT.float32)
        PY = cp.tile([128, NT, K], DT.float32)
        SC = cp.tile([128, NT, K], DT.float32)
        nc.scalar.activation(PX[:], DOTX[:], AF.Square)
        nc.scalar.activation(PY[:], DOTY[:], AF.Square)
        U2 = cp.tile([128, NT, K], DT.float32)
        nc.vector.tensor_tensor(out=U2[:], in0=PX[:], in1=PY[:], op=ALU.add)
        nc.scalar.activation(SC[:], U2[:], AF.Sqrt)
        # one Newton step: s' = 0.5 (s + u/s) makes sqrt correctly-rounded-ish
        RCN = cp.tile([128, NT, K], DT.float32)
        nc.vector.tensor_scalar_max(RCN[:], SC[:], 1e-30)
        nc.vector.reciprocal(RCN[:], RCN[:])
        nc.vector.tensor_tensor(out=RCN[:], in0=U2[:], in1=RCN[:], op=ALU.mult)
        nc.vector.tensor_tensor(out=SC[:], in0=SC[:], in1=RCN[:], op=ALU.add)
        nc.vector.tensor_scalar(out=SC[:], in0=SC[:], scalar1=0.5, scalar2=EPS,
                                op0=ALU.mult, op1=ALU.add)
        nc.vector.reciprocal(SC[:], SC[:])
        nc.vector.tensor_tensor(out=SC[:], in0=SC[:], in1=DD[:], op=ALU.mult)
        nc.vector.tensor_tensor(out=PX[:], in0=DOTX[:], in1=SC[:], op=ALU.mult)
        nc.vector.tensor_tensor(out=PY[:], in0=DOTY[:], in1=SC[:], op=ALU.mult)
        nc.sync.dma_start(pxy_o[:, 0:K].rearrange("(t p) k -> p t k", p=128), PX[:])
        nc.sync.dma_start(pxy_o[:, K:2 * K].rearrange("(t p) k -> p t k", p=128), PY[:])

        # ---- per-cell nearest-3 selection ----
        for t in range(NT):
            pxb = PX[:, t, :].rearrange("p k -> p () k").to_broadcast([128, NCELL, K])
            pyb = PY[:, t, :].rearrange("p k -> p () k").to_broadcast([128, NCELL, K])
            DXT = bp.tile([128, NCELL, K], DT.float32, tag="dx", bufs=3)
            DYT = bp.tile([128, NCELL, K], DT.float32, tag="dy", bufs=3)
            nc.gpsimd.tensor_tensor(out=DXT[:], in0=pxb, in1=TXM[:], op=ALU.subtract)
            nc.vector.tensor_tensor(out=DYT[:], in0=pyb, in1=TYM[:], op=ALU.subtract)
            SQX = bp.tile([128, NCELL, K], DT.float32, tag="sqx", bufs=3)
            SQY = bp.tile([128, NCELL, K], DT.float32, tag="sqy", bufs=3)
            nc.scalar.activation(SQX[:], DXT[:], AF.Square)
            nc.scalar.activation(SQY[:], DYT[:], AF.Square)
            SS = bp.tile([128, NCELL, K], DT.float32, tag="ss", bufs=3)
            nc.gpsimd.tensor_tensor(out=SS[:], in0=SQX[:], in1=SQY[:], op=ALU.add)
            NKEY = bp.tile([128, NCELL, K], DT.float32, tag="nkey", bufs=3)
            nc.vector.scalar_tensor_tensor(
                out=NKEY[:].bitcast(DT.int32), in0=SS[:].bitcast(DT.int32),
                scalar=M32[:], in1=KIOTA[:], op0=ALU.bitwise_and,
                op1=ALU.bitwise_or)
            M8 = bp.tile([128, NCELL, 8], DT.float32, tag="m8", bufs=3)
            for ra in range(NCELL):
                nc.vector.max(out=M8[:, ra, :], in_=NKEY[:, ra, :])
            # contiguous staging copy: a direct DMA of M8[:, :, 0:3] emits
            # 12-byte-element descriptors and runs ~30us each
            M3C = bp.tile([128, NCELL, 3], DT.float32, tag="m3c", bufs=3)
            nc.scalar.activation(M3C[:], M8[:, :, 0:3], AF.Copy)
            nc.sync.dma_start(m3_o[t * 128:(t + 1) * 128, :, :], M3C[:])

    split_sync_waits(nc)
    return nc


# ---------------------------------------------------------------------------
# Host glue
# ---------------------------------------------------------------------------


def host_prep_phase1(vertices):
    """vertices (4, 4096, 3) -> list of 8 input maps."""
    maps = []
    for core in range(8):
        b, h = core // 2, core % 2
        verts = np.ascontiguousarray(vertices[b], dtype=f32)
        sq = (verts * verts).sum(-1, dtype=f32).astype(f32)
        pt4 = np.concatenate([verts.T, sq[None, :]], axis=0).astype(f32)
        Q = verts[h * HALF:(h + 1) * HALF]
        qt4 = np.concatenate([2.0 * Q.T, -np.ones((1, HALF), f32)], axis=0).astype(f32)
        maps.append({"pt4": pt4, "qt4": qt4})
    return maps


_CHUNK_OF_SLOT = (np.arange(CAND) // 8).astype(np.int64)[None, :] * CW


def host_merge(candv, verts, Q):
    """Decode packed candidates, exact fp64 re-rank -> top-33 (d asc, idx asc)."""
    bits = candv.view(np.uint32)
    gidx = _CHUNK_OF_SLOT + (bits & np.uint32(CW - 1)).astype(np.int64)  # (HALF, CAND)
    p = verts.astype(np.float64)[gidx]
    d2 = ((p - Q.astype(np.float64)[:, None, :]) ** 2).sum(-1)
    order = np.lexsort((gidx, d2), axis=1)[:, :33]
    idxs = np.take_along_axis(gidx, order, axis=1)
    d33 = np.sqrt(np.maximum(np.take_along_axis(d2, order, axis=1), 0.0))
    return idxs[:, :32], d33[:, :32].astype(f32), d33[:, 32].astype(f32)


def host_prep_phase2(vertices, template, p1_results):
    """Build phase-2 input maps + per-core nbr tables from phase-1 outputs."""
    template = np.asarray(template, f32)
    tx = template[..., 0].reshape(-1).astype(f32)
    ty = template[..., 1].reshape(-1).astype(f32)
    txy = np.ascontiguousarray(
        np.broadcast_to(np.concatenate([tx, ty])[None, :], (128, 2 * NCELL))
    ).astype(f32)
    maps, nbrs = [], []
    for core in range(8):
        b, h = core // 2, core % 2
        verts = np.ascontiguousarray(vertices[b], dtype=f32)
        Q = verts[h * HALF:(h + 1) * HALF]
        nbr, d, radius = host_merge(p1_results[core]["candv"], verts, Q)
        neigh = (verts[nbr] - Q[:, None, :]).astype(f32)          # (HALF, 32, 3)
        ngh = np.ascontiguousarray(neigh.transpose(0, 2, 1).reshape(HALF, 96))
        w = (radius[:, None] - d).astype(f32)
        wn = (w / (w.sum(1, keepdims=True, dtype=f32) + f32(EPS))).astype(f32)
        wn3 = np.ascontiguousarray(np.tile(wn, (1, 3)))
        maps.append({"ngh": ngh, "wn3": wn3, "dd": np.ascontiguousarray(d),
                     "txy": txy})
        nbrs.append(nbr)
    return maps, nbrs


def host_assemble(p2_results, nbrs, template):
    """Decode closest slots, gather projections, barycentric weights, output."""
    template = np.asarray(template, f32)
    tx = template[..., 0].reshape(-1).astype(f32)[None, :]   # (1, NCELL)
    ty = template[..., 1].reshape(-1).astype(f32)[None, :]
    out = np.zeros((B, V, R, A, 3, 2), f32)
    ar = np.arange(HALF)[:, None, None]
    for core in range(8):
        b, h = core // 2, core % 2
        m3 = np.ascontiguousarray(p2_results[core]["m3o"])        # (HALF, 40, 3)
        k3 = (m3.view(np.int32) & 31).astype(np.int64)            # (HALF, 40, 3)
        pxy = p2_results[core]["pxy"]                             # (HALF, 64)
        px, py = pxy[:, :K], pxy[:, K:]
        x3 = px[ar, k3]                                   # (HALF, 40, 3)
        y3 = py[ar, k3]
        x0, x1, x2 = x3[..., 0], x3[..., 1], x3[..., 2]
        y0, y1, y2 = y3[..., 0], y3[..., 1], y3[..., 2]
        v0x, v0y = x2 - x0, y2 - y0
        v1x, v1y = x1 - x0, y1 - y0
        v2x, v2y = tx - x0, ty - y0
        d00 = v0x * v0x + v0y * v0y
        d01 = v0x * v1x + v0y * v1y
        d02 = v0x * v2x + v0y * v2y
        d11 = v1x * v1x + v1y * v1y
        d12 = v1x * v2x + v1y * v2y
        den = d00 * d11 - d01 * d01 + f32(1e-6)
        w2 = (d11 * d02 - d01 * d12) / den
        w1 = (d00 * d12 - d01 * d02) / den
        w0 = f32(1.0) - w2 - w1
        weights = np.stack([w2, w1, w0], axis=-1)                 # (HALF, 40, 3)
        nbr = nbrs[core]                                          # (HALF, 32)
        pidx = nbr[ar, k3]
        sl = slice(h * HALF, (h + 1) * HALF)
        out[b, sl, ..., 0] = pidx.reshape(HALF, R, A, 3).astype(f32)
        out[b, sl, ..., 1] = weights.reshape(HALF, R, A, 3)
    return out


_PROGS = {}


def _prog(name):
    if name not in _PROGS:
        _PROGS[name] = build_phase1() if name == "p1" else build_phase2()
    return _PROGS[name]


def run_phase1(vertices, trace=False):
    maps = host_prep_phase1(vertices)
    return run_bass_kernel_spmd(_prog("p1"), maps, list(range(8)), trace=trace)


def kernel(vertices, template, trace=False, _timing=None):
    vertices = np.asarray(vertices, f32)
    template = np.asarray(template, f32)
    r1 = run_bass_kernel_spmd(_prog("p1"), host_prep_phase1(vertices),
                              list(range(8)), trace=trace)
    maps2, nbrs = host_prep_phase2(vertices, template, r1.results)
    r2 = run_bass_kernel_spmd(_prog("p2"), maps2, list(range(8)), trace=trace)
    if _timing is not None:
        _timing["phase1"] = r1
        _timing["phase2"] = r2
        _timing["maps2"] = maps2
        _timing["nbrs"] = nbrs
    return host_assemble(r2.results, nbrs, template)


# revision 27
# speedup vs baseline: 1.0101x; 1.0101x over previous
"""Barycentric-coordinates KNN kernel for Trainium2 (8 NeuronCores).

Pipeline (per core = one (batch, half-of-V) pair; 8 cores cover 4 batches x 2 halves):
  Phase 1 (device): per query row, keys -S = 2q.p - |p|^2 (descending rank ==
    ascending d^2) via fp32r (tf32) TensorE matmul; low-7 mantissa bits of the
    PSUM result replaced with the chunk-local index (DVE bitpack STT straight
    from PSUM); DVE max8 per 128-column chunk -> 256 packed candidates/row.
  Host: decode (chunk, k) from packed keys, exact fp64 d^2 re-rank to top-33
    (value asc, index asc), neighbor-coordinate gather, SHOT weight norm.
  Phase 2 (device): weighted 3x3 covariance (fused multiply-accumulate),
    closed-form eigensolver (Newton on the characteristic cubic + cross
    products), SHOT sign disambiguation, tangent-plane log map, template-cell
    nearest-3 selection via bit-packed keys (dist^2 mantissa | k-slot) and
    max8. Ships packed keys + projections.
  Host: decode k-slots, gather projections, barycentric weights,
    pidx = nbr_idx[closest], assemble (4, 4096, 5, 8, 3, 2) output.
"""
import sys

sys.path.insert(0, "/opt/trn_rl_repo")

import numpy as np
from contextlib import ExitStack

import concourse.bass as bass
import concourse.mybir as mybir
import concourse.tile as tile
from concourse.bass_utils import run_bass_kernel_spmd
from concourse.tile import ScopedClock

f32 = np.float32
AF = mybir.ActivationFunctionType
ALU = mybir.AluOpType
DT = mybir.dt

B, V, K = 4, 4096, 32
HALF = V // 2            # queries per core
NT = HALF // 128         # 16 v-tiles per core
CW = 128                 # phase-1 chunk width
NCHUNK = V // CW         # 32 chunks
CAND = NCHUNK * 8        # 256 candidates per row
R, A = 5, 8
NCELL = R * A            # 40 template cells
EPS = 1e-8

# ---------------------------------------------------------------------------
# Tile-framework workaround: walrus rejects instructions carrying more than a
# couple of sync waits. Spread extras across single-wait NOPs.
# ---------------------------------------------------------------------------


def _patched_drain_and_barrier(self, tick_clock, wait_clock):
    probe = self.nc.sync.nop(nofuse=True)
    wait_clock.add_sem_waits(probe.ins, ScopedClock({None: tick_clock.global_clock}))
    sync_info = probe.ins.sync_info
    waits = list(sync_info.on_wait or []) if sync_info is not None else []
    if len(waits) > 1:
        sync_info.on_wait = waits[:1]
        for i in range(1, len(waits)):
            extra = self.nc.sync.nop(nofuse=True)
            if extra.ins.sync_info is None:
                extra.ins.sync_info = mybir.SyncInfo(on_wait=[waits[i]], on_update=[])
            else:
                extra.ins.sync_info.on_wait = [waits[i]]
    self.nc.sync.drain()
    self.nc.all_engine_barrier()
    assert self.sems is not None
    popped = self.nc._tile_sem_poison_stack.pop()
    assert popped is self._sem_poison
    self.nc.clear_and_free_semaphores(list(self.sems.allocated().values()))
    self.nc.all_engine_barrier()


tile.TileContext._drain_and_barrier = _patched_drain_and_barrier


def split_sync_waits(nc, max_waits=1):
    for f in nc.m.functions:
        for b in f.blocks:
            new_list = []
            dirty = False
            for ins in b.instructions:
                si = ins.sync_info
                waits = list(si.on_wait) if (si is not None and si.on_wait) else []
                if len(waits) > max_waits:
                    dirty = True
                    extras, keep = waits[:-max_waits], waits[-max_waits:]
                    for j in range(0, len(extras), max_waits):
                        nop = mybir.InstNoOp(
                            name=f"I-wsplit-{nc.next_id()}", engine=ins.engine
                        )
                        nop.sync_info = mybir.SyncInfo(
                            on_wait=extras[j : j + max_waits], on_update=[]
                        )
                        new_list.append(nop)
                    si.on_wait = keep
                new_list.append(ins)
            if dirty:
                b.instructions = new_list


# ---------------------------------------------------------------------------
# Phase 1 program
# ---------------------------------------------------------------------------


def build_phase1():
    nc = bass.Bass()
    pt4 = nc.declare_dram_parameter("pt4", [4, V], DT.float32r, isOutput=False)
    qt4 = nc.declare_dram_parameter("qt4", [4, HALF], DT.float32r, isOutput=False)
    candv_o = nc.declare_dram_parameter("candv", [HALF, CAND], DT.float32, isOutput=True)

    with tile.TileContext(nc) as tc, ExitStack() as ctx:
        cpool = ctx.enter_context(tc.tile_pool(name="const", bufs=1))
        npool = ctx.enter_context(tc.tile_pool(name="nkey", bufs=3))
        opool = ctx.enter_context(tc.tile_pool(name="cand", bufs=4))
        ppool = ctx.enter_context(tc.tile_pool(name="psum", bufs=2, space="PSUM"))

        pt = cpool.tile([4, V], DT.float32r)
        qt = cpool.tile([4, HALF], DT.float32r)
        nc.sync.dma_start(pt[:], pt4[:])
        nc.sync.dma_start(qt[:], qt4[:])

        KI = cpool.tile([128, 2048], DT.int32)
        nc.gpsimd.iota(KI[:], pattern=[[0, 2048 // CW], [1, CW]], base=0,
                       channel_multiplier=0)
        M128 = cpool.tile([128, 1], DT.int32)
        nc.vector.memset(M128[:], -CW)
        C128 = cpool.tile([128, 1], DT.int32)
        nc.vector.memset(C128[:], CW)

        # Software-pipelined: per tile, jh0 keys are bitpacked on the vector
        # engine (PSUM-direct STT) and consumed by MAX8 immediately; jh1 keys
        # take the slow path (ACT int-quantize -> gpsimd *CW -> gpsimd +iota,
        # ~10us latency) so their MAX8s are deferred one tile to keep the
        # vector engine from stalling on gpsimd.
        NC2 = 2048 // CW
        cvs = [None] * NT
        nks = [None] * NT

        def emit_jh1_max8(t):
            cv, nk1 = cvs[t], nks[t]
            for c in range(NC2):
                cc = NC2 + c
                nc.vector.max(out=cv[:, cc * 8:(cc + 1) * 8],
                              in_=nk1[:, c * CW:(c + 1) * CW].bitcast(DT.float32))
            nc.sync.dma_start(candv_o[t * 128:(t + 1) * 128, :], cv[:])

        for t in range(NT):
            cv = opool.tile([128, CAND], DT.float32, tag="cv")
            cvs[t] = cv
            for jh in range(2):
                ps = ppool.tile([128, 2048], DT.float32, space="PSUM")
                for k4 in range(4):
                    nc.tensor.matmul(
                        ps[:, k4 * 512:(k4 + 1) * 512],
                        qt[:, t * 128:(t + 1) * 128],
                        pt[:, jh * 2048 + k4 * 512: jh * 2048 + (k4 + 1) * 512],
                        start=True, stop=True,
                    )
                nk = npool.tile([128, 2048], DT.int32, tag=f"nk{jh}")
                if jh == 0:
                    nc.vector.scalar_tensor_tensor(
                        out=nk[:], in0=ps[:].bitcast(DT.int32), scalar=M128[:],
                        in1=KI[:], op0=ALU.bitwise_and, op1=ALU.bitwise_or)
                    for c in range(NC2):
                        nc.vector.max(out=cv[:, c * 8:(c + 1) * 8],
                                      in_=nk[:, c * CW:(c + 1) * CW].bitcast(DT.float32))
                else:
                    # gpsimd has no bitwise ops and can't read PSUM. The idle
                    # ACT engine quantizes to a positive int32 rank key
                    # (monotone in the fp32 value, ~5e-7 resolution), gpsimd
                    # shifts it by the chunk width and adds the local index.
                    # The bit layout (v << 7 | k) matches the jh0 half, so
                    # host decode is identical.
                    qv = npool.tile([128, 2048], DT.int32, tag="quant")
                    nc.scalar.activation(qv[:], ps[:], AF.Copy,
                                         bias=4.0 * 2097152.0, scale=2097152.0)
                    sc128 = npool.tile([128, 2048], DT.int32, tag="scaled")
                    nc.gpsimd.tensor_tensor(
                        out=sc128[:], in0=qv[:],
                        in1=C128[:].to_broadcast([128, 2048]), op=ALU.mult)
                    nc.gpsimd.tensor_tensor(out=nk[:], in0=sc128[:], in1=KI[:],
                                            op=ALU.add)
                    nks[t] = nk
            if t > 0:
                emit_jh1_max8(t - 1)
        emit_jh1_max8(NT - 1)

    split_sync_waits(nc)
    return nc


# ---------------------------------------------------------------------------
# Phase 2 program
# ---------------------------------------------------------------------------


def _register_consts(nc, values):
    for value in values:
        t = nc.alloc_sbuf_tensor(f"const-float32-{value}", [128, 1], DT.float32)
        nc.gpsimd.memset(t.ap(), value)
        nc.const_aps.aps[(DT.float32, value)] = t.ap()
    nc.all_engine_barrier()


def build_phase2():
    nc = bass.Bass()
    _register_consts(nc, [0.5])
    ngh_i = nc.declare_dram_parameter("ngh", [128, NT * 96], DT.float32, isOutput=False)
    wn3_i = nc.declare_dram_parameter("wn3", [128, NT * 96], DT.float32, isOutput=False)
    dd_i = nc.declare_dram_parameter("dd", [128, NT * K], DT.float32, isOutput=False)
    txy_i = nc.declare_dram_parameter("txy", [128, 2 * NCELL], DT.float32, isOutput=False)
    m3_o = nc.declare_dram_parameter("m3o", [HALF, NCELL, 3], DT.float32, isOutput=True)
    pxy_o = nc.declare_dram_parameter("pxy", [HALF, 2 * K], DT.float32, isOutput=True)

    with tile.TileContext(nc) as tc, ExitStack() as ctx:
        cp = ctx.enter_context(tc.tile_pool(name="const", bufs=1))
        sp = ctx.enter_context(tc.tile_pool(name="scratch", bufs=2))
        bp = ctx.enter_context(tc.tile_pool(name="bc", bufs=2))

        NGH = cp.tile([128, NT, 96], DT.float32)
        WN3 = cp.tile([128, NT, 96], DT.float32)
        DD = cp.tile([128, NT, K], DT.float32)
        TXY = cp.tile([128, 2 * NCELL], DT.float32)
        nc.sync.dma_start(NGH[:], ngh_i[:].rearrange("p (t c) -> p t c", t=NT))
        nc.sync.dma_start(WN3[:], wn3_i[:].rearrange("p (t c) -> p t c", t=NT))
        nc.sync.dma_start(DD[:], dd_i[:].rearrange("p (t c) -> p t c", t=NT))
        nc.sync.dma_start(TXY[:], txy_i[:])
        TX = TXY[:, 0:NCELL]
        TY = TXY[:, NCELL:2 * NCELL]

        KIOTA = cp.tile([128, NCELL, K], DT.int32)
        nc.gpsimd.iota(KIOTA[:], pattern=[[0, NCELL], [1, K]], base=-2147483648,
                       channel_multiplier=0)
        M32 = cp.tile([128, 1], DT.int32)
        nc.vector.memset(M32[:], -32)
        # materialized template broadcasts: contiguous in1 reads in the
        # per-tile keys loop instead of stride-0 APs
        TXM = cp.tile([128, NCELL, K], DT.float32)
        TYM = cp.tile([128, NCELL, K], DT.float32)
        nc.vector.tensor_copy(
            TXM[:], TX.rearrange("p r -> p r ()").to_broadcast([128, NCELL, K]))
        nc.vector.tensor_copy(
            TYM[:], TY.rearrange("p r -> p r ()").to_broadcast([128, NCELL, K]))

        _tagn = [0]

        def nt_tile(pool=cp):
            _tagn[0] += 1
            return pool.tile([128, NT], DT.float32, tag=f"nt{_tagn[0]}",
                             name=f"nt{_tagn[0]}")

        # ---- covariance accumulation (batched across all v-tiles) ----
        CXX, CXY, CXZ, CYY, CYZ, CZZ = [nt_tile() for _ in range(6)]
        cov_dsts = {"xx": CXX, "xy": CXY, "xz": CXZ, "yy": CYY, "yz": CYZ, "zz": CZZ}
        pairs = [("xx", 0, 0), ("xy", 0, 1), ("xz", 0, 2),
                 ("yy", 1, 1), ("yz", 1, 2), ("zz", 2, 2)]
        NW = sp.tile([128, NT, 96], DT.float32, tag="nw")
        nc.vector.tensor_tensor(out=NW[:], in0=NGH[:], in1=WN3[:], op=ALU.mult)
        for nmq, a, b in pairs:
            prd = sp.tile([128, NT, K], DT.float32, tag="covp")
            nc.vector.tensor_tensor(out=prd[:], in0=NGH[:, :, a * K:(a + 1) * K],
                                    in1=NW[:, :, b * K:(b + 1) * K], op=ALU.mult)
            nc.vector.tensor_reduce(out=cov_dsts[nmq][:], in_=prd[:],
                                    axis=mybir.AxisListType.X, op=ALU.add)

        # ---- eigensolver on (128, NT) ----
        def _ap(x):
            return x if isinstance(x, bass.AP) else x[:]

        def tt(dst, a, bb, op):
            nc.vector.tensor_tensor(out=_ap(dst), in0=_ap(a), in1=_ap(bb), op=op)

        def sq_act(dst, a):
            # vector TT square: keeps the serial eigensolver chain on one
            # engine (ACT round-trips on [128,16] tiles cost ~600ns latency
            # each and stall the chain)
            nc.vector.tensor_tensor(out=dst[:], in0=a[:], in1=a[:], op=ALU.mult)

        Q = nt_tile()
        tt(Q, CXX, CYY, ALU.add)
        tt(Q, Q, CZZ, ALU.add)
        nc.vector.tensor_scalar_mul(Q[:], Q[:], 1.0 / 3.0)
        BXX, BYY, BZZ = nt_tile(), nt_tile(), nt_tile()
        tt(BXX, CXX, Q, ALU.subtract)
        tt(BYY, CYY, Q, ALU.subtract)
        tt(BZZ, CZZ, Q, ALU.subtract)
        P2 = nt_tile()
        T1 = nt_tile(sp)
        sq_act(P2, BXX)
        sq_act(T1, BYY)
        tt(P2, P2, T1, ALU.add)
        sq_act(T1, BZZ)
        tt(P2, P2, T1, ALU.add)
        T2 = nt_tile(sp)
        sq_act(T1, CXY)
        sq_act(T2, CXZ)
        tt(T1, T1, T2, ALU.add)
        sq_act(T2, CYZ)
        tt(T1, T1, T2, ALU.add)
        nc.vector.tensor_scalar_mul(T1[:], T1[:], 2.0)
        tt(P2, P2, T1, ALU.add)
        PP = nt_tile()
        PPX = nt_tile()
        nc.vector.tensor_scalar_mul(PPX[:], P2[:], 1.0 / 6.0)

        def polished_sqrt(dst, x, tmp):
            # ACT Sqrt is ~7e-6; one Newton step s' = (s + x/s)/2 fixes it
            nc.scalar.activation(dst[:], x[:], AF.Sqrt)
            nc.vector.tensor_scalar_max(tmp[:], dst[:], 1e-30)
            nc.vector.reciprocal(tmp[:], tmp[:])
            nc.vector.tensor_tensor(out=tmp[:], in0=x[:], in1=tmp[:], op=ALU.mult)
            nc.vector.tensor_tensor(out=dst[:], in0=dst[:], in1=tmp[:], op=ALU.add)
            nc.vector.tensor_scalar_mul(dst[:], dst[:], 0.5)

        polished_sqrt(PP, PPX, T2)
        PINV = nt_tile()
        nc.vector.tensor_scalar_max(PINV[:], PP[:], 1e-20)
        nc.vector.reciprocal(PINV[:], PINV[:])
        NBXX, NBYY, NBZZ, NBXY, NBXZ, NBYZ = [nt_tile() for _ in range(6)]
        tt(NBXX, BXX, PINV, ALU.mult)
        tt(NBYY, BYY, PINV, ALU.mult)
        tt(NBZZ, BZZ, PINV, ALU.mult)
        tt(NBXY, CXY, PINV, ALU.mult)
        tt(NBXZ, CXZ, PINV, ALU.mult)
        tt(NBYZ, CYZ, PINV, ALU.mult)
        # det(B̂)
        DET = nt_tile()
        sq_act(T1, NBYZ)                     # byz^2
        tt(T2, NBYY, NBZZ, ALU.mult)
        tt(T2, T2, T1, ALU.subtract)
        tt(DET, NBXX, T2, ALU.mult)          # + bxx (byy bzz - byz^2)
        tt(T1, NBXY, NBZZ, ALU.mult)
        tt(T2, NBYZ, NBXZ, ALU.mult)
        tt(T1, T1, T2, ALU.subtract)
        tt(T1, NBXY, T1, ALU.mult)
        tt(DET, DET, T1, ALU.subtract)       # - bxy (bxy bzz - byz bxz)
        tt(T1, NBXY, NBYZ, ALU.mult)
        tt(T2, NBYY, NBXZ, ALU.mult)
        tt(T1, T1, T2, ALU.subtract)
        tt(T1, NBXZ, T1, ALU.mult)
        tt(DET, DET, T1, ALU.add)            # + bxz (bxy byz - byy bxz)
        R2 = nt_tile()                       # 2r = det  clamped to [-2, 2]
        nc.vector.tensor_scalar_min(R2[:], DET[:], 2.0)
        nc.vector.tensor_scalar_max(R2[:], R2[:], -2.0)

        # Both Newton chains (min/max root) and both eigenvector extractions
        # run identical code on different data; batch each pair into
        # [128, 2, NT] tiles (slice 0 = min root / Z axis, 1 = max / X).
        _tag2 = [0]

        def nt2_tile(pool=cp):
            _tag2[0] += 1
            return pool.tile([128, 2, NT], DT.float32, tag=f"n2_{_tag2[0]}",
                             name=f"n2_{_tag2[0]}")

        def b2(x):
            return _ap(x).rearrange("p t -> p () t").to_broadcast([128, 2, NT])

        BETA = nt2_tile()
        nc.vector.memset(BETA[:, 0, :], -2.2)
        nc.vector.memset(BETA[:, 1, :], 2.2)
        R2B = b2(R2)
        FV = nt2_tile(sp)
        B2T = nt2_tile(sp)
        T1B = nt2_tile(sp)
        for _ in range(8):
            # β' = β - f/f' = (2β³ + 2r) / (3β² - 3)
            sq_act(B2T, BETA)                             # β²
            tt(FV, B2T, BETA, ALU.mult)                   # β³
            nc.vector.scalar_tensor_tensor(
                out=T1B[:], in0=FV[:], scalar=2.0, in1=R2B,
                op0=ALU.mult, op1=ALU.add)                # 2β³ + 2r
            nc.vector.tensor_scalar(out=B2T[:], in0=B2T[:], scalar1=3.0,
                                    scalar2=-3.0, op0=ALU.mult, op1=ALU.add)  # f' = 3β²-3
            nc.vector.tensor_scalar_max(B2T[:], B2T[:], 1e-8)
            nc.vector.reciprocal(B2T[:], B2T[:])
            tt(BETA, T1B, B2T, ALU.mult)                  # β'
        LAM = nt2_tile()
        tt(LAM, b2(PP), BETA, ALU.mult)
        tt(LAM, LAM, b2(Q), ALU.add)

        # columns of A - lam I (cov components broadcast over the pair dim)
        D0, D1, D2 = nt2_tile(sp), nt2_tile(sp), nt2_tile(sp)
        tt(D0, b2(CXX), LAM, ALU.subtract)
        tt(D1, b2(CYY), LAM, ALU.subtract)
        tt(D2, b2(CZZ), LAM, ALU.subtract)
        m0 = (D0, b2(CXY), b2(CXZ))
        m1 = (b2(CXY), D1, b2(CYZ))
        m2 = (b2(CXZ), b2(CYZ), D2)

        def cross(u, v):
            rx, ry, rz = nt2_tile(sp), nt2_tile(sp), nt2_tile(sp)
            tt(rx, u[1], v[2], ALU.mult)
            tt(T1B, u[2], v[1], ALU.mult)
            tt(rx, rx, T1B, ALU.subtract)
            tt(ry, u[2], v[0], ALU.mult)
            tt(T1B, u[0], v[2], ALU.mult)
            tt(ry, ry, T1B, ALU.subtract)
            tt(rz, u[0], v[1], ALU.mult)
            tt(T1B, u[1], v[0], ALU.mult)
            tt(rz, rz, T1B, ALU.subtract)
            return rx, ry, rz

        def norm2(c):
            n = nt2_tile(sp)
            sq_act(n, c[0])
            sq_act(T1B, c[1])
            tt(n, n, T1B, ALU.add)
            sq_act(T1B, c[2])
            tt(n, n, T1B, ALU.add)
            return n

        c01 = cross(m0, m1)
        c02 = cross(m0, m2)
        c12 = cross(m1, m2)
        n01, n02, n12 = norm2(c01), norm2(c02), norm2(c12)
        G1, G2, G3 = nt2_tile(sp), nt2_tile(sp), nt2_tile(sp)
        tt(G1, n01, n02, ALU.is_ge)
        tt(G2, n01, n12, ALU.is_ge)
        tt(G1, G1, G2, ALU.mult)                    # pick01
        tt(G3, n02, n12, ALU.is_ge)
        U = nt2_tile(sp)
        nc.vector.tensor_scalar(out=U[:], in0=G1[:], scalar1=-1.0, scalar2=1.0,
                                op0=ALU.mult, op1=ALU.add)   # 1 - pick01
        tt(G2, U, G3, ALU.mult)                     # pick02
        nc.vector.tensor_scalar(out=G3[:], in0=G3[:], scalar1=-1.0, scalar2=1.0,
                                op0=ALU.mult, op1=ALU.add)   # 1 - g3
        tt(G3, U, G3, ALU.mult)                     # pick12
        EV = []
        for ci in range(3):
            VC = nt2_tile()
            tt(VC, c01[ci], G1, ALU.mult)
            tt(T1B, c02[ci], G2, ALU.mult)
            tt(VC, VC, T1B, ALU.add)
            tt(T1B, c12[ci], G3, ALU.mult)
            tt(VC, VC, T1B, ALU.add)
            EV.append(VC)
        n2v = norm2(EV)
        nrm = nt2_tile(sp)
        polished_sqrt(nrm, n2v, T1B)
        nc.vector.tensor_scalar_max(nrm[:], nrm[:], 1e-30)
        nc.vector.reciprocal(nrm[:], nrm[:])
        for VC in EV:
            tt(VC, VC, nrm, ALU.mult)
        ZAX = [EV[c][:, 0, :] for c in range(3)]
        XAX = [EV[c][:, 1, :] for c in range(3)]

        # ---- disambiguation dots (batched: axis scalar broadcast over k) ----
        def batched_dot(DST, AX):
            T = sp.tile([128, NT, K], DT.float32, tag="dotT")
            for c in range(3):
                axb = _ap(AX[c]).rearrange("p t -> p t ()").to_broadcast([128, NT, K])
                dst = DST[:] if c == 0 else T[:]
                nc.vector.tensor_tensor(out=dst, in0=NGH[:, :, c * K:(c + 1) * K],
                                        in1=axb, op=ALU.mult)
                if c > 0:
                    nc.vector.tensor_tensor(out=DST[:], in0=DST[:], in1=T[:],
                                            op=ALU.add)

        DOTX = cp.tile([128, NT, K], DT.float32)
        DOTZ = cp.tile([128, NT, K], DT.float32)
        batched_dot(DOTX, XAX)
        batched_dot(DOTZ, ZAX)

        SG = cp.tile([128, NT, K], DT.float32)
        FX = nt_tile()
        FZ = nt_tile()
        for DOT, F in ((DOTX, FX), (DOTZ, FZ)):
            nc.scalar.activation(SG[:], DOT[:], AF.Sign)
            nc.vector.tensor_reduce(out=F[:], in_=SG[:], axis=mybir.AxisListType.X,
                                    op=ALU.add)
            nc.scalar.activation(F[:], F[:], AF.Sign, bias=0.5, scale=1.0)
        for c in range(3):
            tt(XAX[c], XAX[c], FX, ALU.mult)
            tt(ZAX[c], ZAX[c], FZ, ALU.mult)
        fxb = FX[:].rearrange("p t -> p t ()").to_broadcast([128, NT, K])
        nc.vector.tensor_tensor(out=DOTX[:], in0=DOTX[:], in1=fxb, op=ALU.mult)
        # y = cross(z, x)
        YAX = []
        for (i1, i2) in ((1, 2), (2, 0), (0, 1)):
            YC = nt_tile()
            tt(YC, ZAX[i1], XAX[i2], ALU.mult)
            tt(T1, ZAX[i2], XAX[i1], ALU.mult)
            tt(YC, YC, T1, ALU.subtract)
            YAX.append(YC)
        DOTY = cp.tile([128, NT, K], DT.float32)
        batched_dot(DOTY, YAX)

        # ---- projections (batched over all tiles) ----
        PX = cp.tile([128, NT, K], DT.float32)
        PY = cp.tile([128, NT, K], DT.float32)
        SC = cp.tile([128, NT, K], DT.float32)
        nc.scalar.activation(PX[:], DOTX[:], AF.Square)
        nc.scalar.activation(PY[:], DOTY[:], AF.Square)
        U2 = cp.tile([128, NT, K], DT.float32)
        nc.vector.tensor_tensor(out=U2[:], in0=PX[:], in1=PY[:], op=ALU.add)
        nc.scalar.activation(SC[:], U2[:], AF.Sqrt)
        # one Newton step: s' = 0.5 (s + u/s) makes sqrt correctly-rounded-ish
        RCN = cp.tile([128, NT, K], DT.float32)
        nc.vector.tensor_scalar_max(RCN[:], SC[:], 1e-30)
        nc.vector.reciprocal(RCN[:], RCN[:])
        nc.vector.tensor_tensor(out=RCN[:], in0=U2[:], in1=RCN[:], op=ALU.mult)
        nc.vector.tensor_tensor(out=SC[:], in0=SC[:], in1=RCN[:], op=ALU.add)
        nc.vector.tensor_scalar(out=SC[:], in0=SC[:], scalar1=0.5, scalar2=EPS,
                                op0=ALU.mult, op1=ALU.add)
        nc.vector.reciprocal(SC[:], SC[:])
        nc.vector.tensor_tensor(out=SC[:], in0=SC[:], in1=DD[:], op=ALU.mult)
        nc.vector.tensor_tensor(out=PX[:], in0=DOTX[:], in1=SC[:], op=ALU.mult)
        nc.vector.tensor_tensor(out=PY[:], in0=DOTY[:], in1=SC[:], op=ALU.mult)
        nc.sync.dma_start(pxy_o[:, 0:K].rearrange("(t p) k -> p t k", p=128), PX[:])
        nc.sync.dma_start(pxy_o[:, K:2 * K].rearrange("(t p) k -> p t k", p=128), PY[:])

        # ---- per-cell nearest-3 selection ----
        for t in range(NT):
            pxb = PX[:, t, :].rearrange("p k -> p () k").to_broadcast([128, NCELL, K])
            pyb = PY[:, t, :].rearrange("p k -> p () k").to_broadcast([128, NCELL, K])
            DXT = bp.tile([128, NCELL, K], DT.float32, tag="dx", bufs=3)
            DYT = bp.tile([128, NCELL, K], DT.float32, tag="dy", bufs=3)
            nc.gpsimd.tensor_tensor(out=DXT[:], in0=pxb, in1=TXM[:], op=ALU.subtract)
            nc.vector.tensor_tensor(out=DYT[:], in0=pyb, in1=TYM[:], op=ALU.subtract)
            SQX = bp.tile([128, NCELL, K], DT.float32, tag="sqx", bufs=3)
            SQY = bp.tile([128, NCELL, K], DT.float32, tag="sqy", bufs=3)
            nc.scalar.activation(SQX[:], DXT[:], AF.Square)
            nc.scalar.activation(SQY[:], DYT[:], AF.Square)
            SS = bp.tile([128, NCELL, K], DT.float32, tag="ss", bufs=3)
            nc.gpsimd.tensor_tensor(out=SS[:], in0=SQX[:], in1=SQY[:], op=ALU.add)
            NKEY = bp.tile([128, NCELL, K], DT.float32, tag="nkey", bufs=3)
            nc.vector.scalar_tensor_tensor(
                out=NKEY[:].bitcast(DT.int32), in0=SS[:].bitcast(DT.int32),
                scalar=M32[:], in1=KIOTA[:], op0=ALU.bitwise_and,
                op1=ALU.bitwise_or)
            M8 = bp.tile([128, NCELL, 8], DT.float32, tag="m8", bufs=3)
            for ra in range(NCELL):
                nc.vector.max(out=M8[:, ra, :], in_=NKEY[:, ra, :])
            # contiguous staging copy: a direct DMA of M8[:, :, 0:3] emits
            # 12-byte-element descriptors and runs ~30us each
            M3C = bp.tile([128, NCELL, 3], DT.float32, tag="m3c", bufs=3)
            nc.scalar.activation(M3C[:], M8[:, :, 0:3], AF.Copy)
            nc.sync.dma_start(m3_o[t * 128:(t + 1) * 128, :, :], M3C[:])

    split_sync_waits(nc)
    return nc


# ---------------------------------------------------------------------------
# Host glue
# ---------------------------------------------------------------------------


def host_prep_phase1(vertices):
    """vertices (4, 4096, 3) -> list of 8 input maps."""
    maps = []
    for core in range(8):
        b, h = core // 2, core % 2
        verts = np.ascontiguousarray(vertices[b], dtype=f32)
        sq = (verts * verts).sum(-1, dtype=f32).astype(f32)
        pt4 = np.concatenate([verts.T, sq[None, :]], axis=0).astype(f32)
        Q = verts[h * HALF:(h + 1) * HALF]
        qt4 = np.concatenate([2.0 * Q.T, -np.ones((1, HALF), f32)], axis=0).astype(f32)
        maps.append({"pt4": pt4, "qt4": qt4})
    return maps


_CHUNK_OF_SLOT = (np.arange(CAND) // 8).astype(np.int64)[None, :] * CW


def host_merge(candv, verts, Q):
    """Decode packed candidates, exact fp64 re-rank -> top-33 (d asc, idx asc)."""
    bits = candv.view(np.uint32)
    gidx = _CHUNK_OF_SLOT + (bits & np.uint32(CW - 1)).astype(np.int64)  # (HALF, CAND)
    p = verts.astype(np.float64)[gidx]
    d2 = ((p - Q.astype(np.float64)[:, None, :]) ** 2).sum(-1)
    order = np.lexsort((gidx, d2), axis=1)[:, :33]
    idxs = np.take_along_axis(gidx, order, axis=1)
    d33 = np.sqrt(np.maximum(np.take_along_axis(d2, order, axis=1), 0.0))
    return idxs[:, :32], d33[:, :32].astype(f32), d33[:, 32].astype(f32)


def host_prep_phase2(vertices, template, p1_results):
    """Build phase-2 input maps + per-core nbr tables from phase-1 outputs."""
    template = np.asarray(template, f32)
    tx = template[..., 0].reshape(-1).astype(f32)
    ty = template[..., 1].reshape(-1).astype(f32)
    txy = np.ascontiguousarray(
        np.broadcast_to(np.concatenate([tx, ty])[None, :], (128, 2 * NCELL))
    ).astype(f32)
    maps, nbrs = [], []
    for core in range(8):
        b, h = core // 2, core % 2
        verts = np.ascontiguousarray(vertices[b], dtype=f32)
        Q = verts[h * HALF:(h + 1) * HALF]
        nbr, d, radius = host_merge(p1_results[core]["candv"], verts, Q)
        neigh = (verts[nbr] - Q[:, None, :]).astype(f32)          # (HALF, 32, 3)
        ngh = neigh.transpose(0, 2, 1).reshape(HALF, 96)
        w = (radius[:, None] - d).astype(f32)
        wn = (w / (w.sum(1, keepdims=True, dtype=f32) + f32(EPS))).astype(f32)
        wn3 = np.tile(wn, (1, 3))

        def dev_t(a):
            # [HALF, c] -> [128, NT*c]: partition-major layout, contiguous DMA
            c = a.shape[1]
            return np.ascontiguousarray(
                a.reshape(NT, 128, c).transpose(1, 0, 2).reshape(128, NT * c))

        maps.append({"ngh": dev_t(ngh), "wn3": dev_t(wn3), "dd": dev_t(d),
                     "txy": txy})
        nbrs.append(nbr)
    return maps, nbrs


def host_assemble(p2_results, nbrs, template):
    """Decode closest slots, gather projections, barycentric weights, output."""
    template = np.asarray(template, f32)
    tx = template[..., 0].reshape(-1).astype(f32)[None, :]   # (1, NCELL)
    ty = template[..., 1].reshape(-1).astype(f32)[None, :]
    out = np.zeros((B, V, R, A, 3, 2), f32)
    ar = np.arange(HALF)[:, None, None]
    for core in range(8):
        b, h = core // 2, core % 2
        m3 = np.ascontiguousarray(p2_results[core]["m3o"])        # (HALF, 40, 3)
        k3 = (m3.view(np.int32) & 31).astype(np.int64)            # (HALF, 40, 3)
        pxy = p2_results[core]["pxy"]                             # (HALF, 64)
        px, py = pxy[:, :K], pxy[:, K:]
        x3 = px[ar, k3]                                   # (HALF, 40, 3)
        y3 = py[ar, k3]
        x0, x1, x2 = x3[..., 0], x3[..., 1], x3[..., 2]
        y0, y1, y2 = y3[..., 0], y3[..., 1], y3[..., 2]
        v0x, v0y = x2 - x0, y2 - y0
        v1x, v1y = x1 - x0, y1 - y0
        v2x, v2y = tx - x0, ty - y0
        d00 = v0x * v0x + v0y * v0y
        d01 = v0x * v1x + v0y * v1y
        d02 = v0x * v2x + v0y * v2y
        d11 = v1x * v1x + v1y * v1y
        d12 = v1x * v2x + v1y * v2y
        den = d00 * d11 - d01 * d01 + f32(1e-6)
        w2 = (d11 * d02 - d01 * d12) / den
        w1 = (d00 * d12 - d01 * d02) / den
        w0 = f32(1.0) - w2 - w1
        weights = np.stack([w2, w1, w0], axis=-1)                 # (HALF, 40, 3)
        nbr = nbrs[core]                                          # (HALF, 32)
        pidx = nbr[ar, k3]
        sl = slice(h * HALF, (h + 1) * HALF)
        out[b, sl, ..., 0] = pidx.reshape(HALF, R, A, 3).astype(f32)
        out[b, sl, ..., 1] = weights.reshape(HALF, R, A, 3)
    return out


_PROGS = {}


def _prog(name):
    if name not in _PROGS:
        _PROGS[name] = build_phase1() if name == "p1" else build_phase2()
    return _PROGS[name]


def run_phase1(vertices, trace=False):
    maps = host_prep_phase1(vertices)
    return run_bass_kernel_spmd(_prog("p1"), maps, list(range(8)), trace=trace)


def kernel(vertices, template, trace=False, _timing=None):
    vertices = np.asarray(vertices, f32)
    template = np.asarray(template, f32)
    r1 = run_bass_kernel_spmd(_prog("p1"), host_prep_phase1(vertices),
                              list(range(8)), trace=trace)
    maps2, nbrs = host_prep_phase2(vertices, template, r1.results)
    r2 = run_bass_kernel_spmd(_prog("p2"), maps2, list(range(8)), trace=trace)
    if _timing is not None:
        _timing["phase1"] = r1
        _timing["phase2"] = r2
        _timing["maps2"] = maps2
        _timing["nbrs"] = nbrs
    return host_assemble(r2.results, nbrs, template)
